# revision 12
# baseline (speedup 1.0000x reference)
"""AblationCAM Trainium2 kernel (8 NeuronCores, SPMD via bass/Tile).

Math restructure (identical function to the reference):
  - maxpool commutes with channel ablation (inputs are non-negative), so
    ablated flat vectors are f with segment i (49 values) zeroed.
  - h1_abl[i] = relu(z1 - delta_i), delta[i, j1] = sum_{k in seg_i} W1T[k, j1] f[k]
  - only column `img_class` of the ablated logits is needed:
      numer[i] = w3c . (relu(z2) - relu(z2 - D2[i])),  D2 = D1 @ W2^T
      alpha[i] = numer[i] / base
  - the saliency fuse/resize/normalize is a tiny (14x14 -> 224x224) linear map
    done on the host, exactly replicating jax.image.resize bilinear weights.

Sharding: W1^T row-sharded over j1 (each core reads its 51.4MB W1 slice once);
W2^T/W3 column-sharded over j2; D1 (8.4MB) AllGathered; logits/numer
AllReduced (tiny).
"""

from contextlib import ExitStack

import numpy as np

import concourse.bacc as bacc
import concourse.bass as bass
import concourse.mybir as mybir
import concourse.tile as tile
from concourse.bass_utils import run_bass_kernel_spmd
from concourse.masks import make_identity

F32 = mybir.dt.float32
RELU = mybir.ActivationFunctionType.Relu
ADD = mybir.AluOpType.add
SUB = mybir.AluOpType.subtract
MULT = mybir.AluOpType.mult

NCORES = 8
C = 512            # channels == number of ablations
S = 49             # 7*7 flattened spatial per channel
K = C * S          # 25088 contraction dim of layer 1
FC = 4096
NCLS = 1000
J1 = FC // NCORES  # 512: per-core layer-1 rows
J2 = FC // NCORES  # 512: per-core layer-2 outputs
IB = 128           # channels per i-block group
NG = C // IB       # 4 groups
GT = S             # 49 k-chunks (of 128 rows) per group
SUP = 7            # k-chunks per w1t super-tile DMA
NU = FC // 128     # 32 j1-chunks

_CACHE = {}


def _emit(nc, tc, t, debug=False, stage=99):
    es = ExitStack()
    sb = es.enter_context(tc.tile_pool(name="sb", bufs=1))
    sb2 = es.enter_context(tc.tile_pool(name="sb2", bufs=2))
    sb3 = es.enter_context(tc.tile_pool(name="sb3", bufs=3))
    # PSUM budget is 8 banks of [128, 512]f32:
    #   scratch x2 + acc x2 + d2 x4
    ps = es.enter_context(tc.tile_pool(name="ps", bufs=2, space="PSUM"))
    psacc = es.enter_context(tc.tile_pool(name="psacc", bufs=2, space="PSUM"))
    psd2 = es.enter_context(tc.tile_pool(name="psd2", bufs=1, space="PSUM"))

    # ---------------- constants ----------------
    ident = sb.tile([128, 128], F32, tag="ident")
    make_identity(nc, ident[:])
    ones_col = sb.tile([128, 1], F32, tag="ones_col")
    nc.gpsimd.memset(ones_col[:], 1.0)
    ones_row = sb.tile([1, 128], F32, tag="ones_row")
    nc.gpsimd.memset(ones_row[:], 1.0)
    one_one = sb.tile([1, 1], F32, tag="one_one")
    nc.gpsimd.memset(one_one[:], 1.0)

    # ---------------- small inputs ----------------
    fch = sb.tile([128, K // 128], F32, tag="fch")       # f[128u + p] at [p, u]
    nc.sync.dma_start(out=fch[:], in_=t["fch"][:, :])
    b1s = sb.tile([1, J1], F32, tag="b1s")
    nc.sync.dma_start(out=b1s[:], in_=t["b1s"][:, :])
    b2s = sb.tile([1, J2], F32, tag="b2s")
    nc.sync.dma_start(out=b2s[:], in_=t["b2s"][:, :])
    b3f = sb.tile([1, NCLS], F32, tag="b3f")
    nc.sync.dma_start(out=b3f[:], in_=t["b3f"][:, :])
    # W3 natural [1000, 512]: 8 row-tiles (c on partitions)
    w3rows = [(128 * q, min(128, NCLS - 128 * q)) for q in range(8)]
    w3n = []
    for q, (r0, pr) in enumerate(w3rows):
        w3n_q = sb.tile([128, J2], F32, tag=f"w3n{q}")
        nc.sync.dma_start(out=w3n_q[:pr, :], in_=t["w3n"][r0:r0 + pr, :])
        w3n.append(w3n_q)

    # ---------------- W1 stage: delta + z1 ----------------
    # sf[p, tt, c] = f[128(49 g + tt) + p] if c == floor((128 tt + p)/49) else 0
    delta_sb = []
    z1_ps = psacc.tile([1, J1], F32, tag="acc")
    for g in range(NG):
        sf = sb2.tile([128, GT, IB], F32, tag="sf")
        fg = fch[:, GT * g:GT * (g + 1)].unsqueeze(2).broadcast_to((128, GT, IB))
        # keep where p + 128 tt - 49 c >= 0
        nc.gpsimd.affine_select(
            out=sf[:], in_=fg, compare_op=mybir.AluOpType.is_ge, fill=0.0,
            base=0, pattern=[[128, GT], [-49, IB]], channel_multiplier=1)
        # keep where 48 - p - 128 tt + 49 c >= 0
        nc.gpsimd.affine_select(
            out=sf[:], in_=sf[:], compare_op=mybir.AluOpType.is_ge, fill=0.0,
            base=48, pattern=[[-128, GT], [49, IB]], channel_multiplier=-1)
        d_ps = ps.tile([128, J1], F32, tag="scratch")
        for sup in range(GT // SUP):
            w1sup = sb2.tile([128, SUP, J1], F32, tag="w1sup")
            r0 = 128 * (GT * g + SUP * sup)
            nc.sync.dma_start(
                out=w1sup[:],
                in_=t["w1t"][r0:r0 + 128 * SUP, :].rearrange(
                    "(q p) j -> p q j", p=128, q=SUP))
            for q in range(SUP):
                tt = SUP * sup + q
                nc.tensor.matmul(
                    d_ps[:], sf[:, tt, :], w1sup[:, q, :],
                    start=(tt == 0), stop=(tt == GT - 1))
        d_g = sb.tile([128, J1], F32, tag=f"delta{g}")
        nc.vector.tensor_copy(d_g[:], d_ps[:])
        delta_sb.append(d_g)
        # z1 partial: sum over the 128 channels of this group
        nc.tensor.matmul(z1_ps[:], ones_col[:], d_g[:],
                         start=(g == 0), stop=(g == NG - 1))

    # z1 = sum_i delta + b1 ; h1 = relu(z1)
    z1 = sb.tile([1, J1], F32, tag="z1")
    nc.vector.tensor_tensor(out=z1[:], in0=z1_ps[:], in1=b1s[:], op=ADD)
    h1 = sb.tile([1, J1], F32, tag="h1")
    nc.scalar.activation(h1[:], z1[:], RELU)
    if debug:
        nc.sync.dma_start(out=t["z1_dbg"][:, :], in_=z1[:])
        nc.sync.dma_start(out=t["delta_dbg"][:, :], in_=delta_sb[0][:])

    if stage <= 1:
        es.close()
        return

    # broadcast z1 across partitions: [128, J1]
    z1b_ps = ps.tile([128, J1], F32, tag="scratch")
    nc.tensor.matmul(z1b_ps[:], ones_row[:], z1[:], start=True, stop=True)
    z1b = sb.tile([128, J1], F32, tag="z1b")
    nc.scalar.copy(z1b[:], z1b_ps[:])
    rz1b = sb.tile([128, J1], F32, tag="rz1b")
    nc.scalar.activation(rz1b[:], z1b[:], RELU)

    # d1n[i, j1] = relu(z1 - delta_i) - relu(z1)   (= h1_abl - h1_base)
    # transpose to [j1, i] and write to dram for the allgather
    for g in range(NG):
        t1 = sb2.tile([128, J1], F32, tag="ta")
        nc.vector.tensor_tensor(out=t1[:], in0=z1b[:], in1=delta_sb[g][:], op=SUB)
        r1 = sb2.tile([128, J1], F32, tag="tb")
        nc.scalar.activation(r1[:], t1[:], RELU)
        d1n_g = sb2.tile([128, J1], F32, tag="tc")
        nc.vector.tensor_tensor(out=d1n_g[:], in0=r1[:], in1=rz1b[:], op=SUB)
        for u in range(J1 // 128):
            tp = ps.tile([128, 128], F32, tag="scratch")
            nc.tensor.transpose(tp[:], d1n_g[:, 128 * u:128 * (u + 1)], ident[:])
            ev = sb2.tile([128, 128], F32, tag="tpe")
            nc.scalar.copy(ev[:], tp[:])
            nc.sync.dma_start(
                out=t["d1nt_dram"][128 * u:128 * (u + 1),
                                   128 * g:128 * (g + 1)],
                in_=ev[:])

    nc.gpsimd.collective_compute(
        "AllGather", mybir.AluOpType.bypass,
        replica_groups=[list(range(NCORES))],
        ins=[t["d1nt_dram"][:, :].opt()], outs=[t["d1full_dram"][:, :].opt()])

    # ---------------- h1 gather, z2 pass ----------------
    nc.sync.dma_start(out=t["h1_dram"][:, :], in_=h1[:])
    nc.gpsimd.collective_compute(
        "AllGather", mybir.AluOpType.bypass,
        replica_groups=[list(range(NCORES))],
        ins=[t["h1_dram"][:, :].opt()], outs=[t["h1full_dram"][:, :].opt()])
    h1nat = sb.tile([32, 128], F32, tag="h1nat")
    nc.sync.dma_start(
        out=h1nat[:],
        in_=t["h1full_dram"][:, :].rearrange("a (b c) -> (a b) c", c=128))
    h1t_ps = ps.tile([128, 32], F32, tag="scratch")
    nc.tensor.transpose(h1t_ps[:], h1nat[:], ident[:32, :32])
    h1t = sb.tile([128, 32], F32, tag="h1t")
    nc.scalar.copy(h1t[:], h1t_ps[:])

    z2_ps = psacc.tile([1, J2], F32, tag="acc")
    for u in range(NU):
        w2t_u = sb3.tile([128, J2], F32, tag="w2ta")
        nc.sync.dma_start(out=w2t_u[:], in_=t["w2t"][128 * u:128 * (u + 1), :])
        nc.tensor.matmul(z2_ps[:], h1t[:, u:u + 1], w2t_u[:],
                         start=(u == 0), stop=(u == NU - 1))
    z2 = sb.tile([1, J2], F32, tag="z2")
    nc.vector.tensor_tensor(out=z2[:], in0=z2_ps[:], in1=b2s[:], op=ADD)
    z2b_ps = ps.tile([128, J2], F32, tag="scratch")
    nc.tensor.matmul(z2b_ps[:], ones_row[:], z2[:], start=True, stop=True)
    z2b = sb.tile([128, J2], F32, tag="z2b")
    nc.scalar.copy(z2b[:], z2b_ps[:])
    rz2b = sb.tile([128, J2], F32, tag="rz2b")
    nc.scalar.activation(rz2b[:], z2b[:], RELU)
    if debug:
        h2dbg = sb.tile([1, J2], F32, tag="h2dbg")
        nc.scalar.activation(h2dbg[:], z2[:], RELU)
        nc.sync.dma_start(out=t["h2_dbg"][:, :], in_=h2dbg[:])

    if stage <= 2:
        es.close()
        return

    # ---------------- logits via row-dot TTR ----------------
    lgT = sb.tile([128, 8], F32, tag="lgT")
    for q, (r0, pr) in enumerate(w3rows):
        prod = sb2.tile([128, J2], F32, tag="td")
        nc.vector.tensor_tensor(out=prod[:pr, :], in0=w3n[q][:pr, :],
                                in1=rz2b[:pr, :], op=MULT)
        nc.vector.tensor_reduce(out=lgT[:pr, q:q + 1], in_=prod[:pr, :],
                                axis=mybir.AxisListType.X, op=ADD)
    lga = ps.tile([1, 512], F32, tag="scratch")
    lgb_ps = ps.tile([1, 512], F32, tag="scratch")
    for q, (r0, pr) in enumerate(w3rows):
        dst = lga if q < 4 else lgb_ps
        col = 128 * q if q < 4 else 128 * (q - 4)
        nc.tensor.transpose(dst[:, col:col + pr], lgT[:pr, q:q + 1],
                            ident[:pr, :pr])
    lg = sb.tile([1, NCLS], F32, tag="lg")
    nc.scalar.copy(lg[:, 0:512], lga[:])
    nc.scalar.copy(lg[:, 512:NCLS], lgb_ps[:, 0:NCLS - 512])
    nc.sync.dma_start(out=t["lg_in"][:, :], in_=lg[:])
    nc.gpsimd.collective_compute(
        "AllReduce", ADD,
        replica_groups=[list(range(NCORES))],
        ins=[t["lg_in"][:, :].opt()], outs=[t["lg_out"][:, :].opt()])
    lgf = sb.tile([1, NCLS], F32, tag="lgf")
    nc.sync.dma_start(out=lgf[:], in_=t["lg_out"][:, :])
    lgb = sb.tile([1, NCLS], F32, tag="lgb")
    nc.vector.tensor_tensor(out=lgb[:], in0=lgf[:], in1=b3f[:], op=ADD)
    nc.sync.dma_start(out=t["logits_out"][:, :], in_=lgb[:])

    # ---------------- one-hot argmax -> w3c ----------------
    mx = sb.tile([1, 1], F32, tag="mx")
    nc.vector.tensor_reduce(out=mx[:], in_=lgb[:], axis=mybir.AxisListType.X,
                            op=mybir.AluOpType.max)
    oh = sb.tile([1, NCLS], F32, tag="oh")
    nc.vector.tensor_scalar(out=oh[:], in0=lgb[:], scalar1=mx[:], scalar2=None,
                            op0=mybir.AluOpType.is_equal)
    ohT_ps = ps.tile([128, 8], F32, tag="scratch")
    for q, (r0, pr) in enumerate(w3rows):
        nc.tensor.transpose(ohT_ps[:pr, q:q + 1], oh[:, r0:r0 + pr], one_one[:])
    ohT = sb.tile([128, 8], F32, tag="ohT")
    for q, (r0, pr) in enumerate(w3rows):
        nc.scalar.copy(ohT[:pr, q:q + 1], ohT_ps[:pr, q:q + 1])
    w3c_ps = psacc.tile([1, J2], F32, tag="acc")
    for q, (r0, pr) in enumerate(w3rows):
        nc.tensor.matmul(w3c_ps[:], ohT[:pr, q:q + 1], w3n[q][:pr, :],
                         start=(q == 0), stop=(q == 7))
    w3c = sb.tile([1, J2], F32, tag="w3c")
    nc.scalar.copy(w3c[:], w3c_ps[:])
    w3cb_ps = ps.tile([128, J2], F32, tag="scratch")
    nc.tensor.matmul(w3cb_ps[:], ones_row[:], w3c[:], start=True, stop=True)
    w3cb = sb.tile([128, J2], F32, tag="w3cb")
    nc.scalar.copy(w3cb[:], w3cb_ps[:])

    if stage <= 3:
        es.close()
        return

    # ---------------- D2 pass ----------------
    d2_ps = [psd2.tile([128, J2], F32, tag=f"d2ps{it}", name=f"d2ps{it}")
             for it in range(4)]
    for u in range(NU):
        d1f_u = sb3.tile([128, C], F32, tag="d1f")
        nc.sync.dma_start(out=d1f_u[:],
                          in_=t["d1full_dram"][128 * u:128 * (u + 1), :])
        w2t_u = sb3.tile([128, J2], F32, tag="w2tb")
        nc.sync.dma_start(out=w2t_u[:], in_=t["w2t"][128 * u:128 * (u + 1), :])
        for it in range(4):
            nc.tensor.matmul(d2_ps[it][:], d1f_u[:, 128 * it:128 * (it + 1)],
                             w2t_u[:], start=(u == 0), stop=(u == NU - 1))

    # ---------------- R + numer ----------------
    numt = sb.tile([128, 4], F32, tag="numt")
    for it in range(4):
        t2 = sb2.tile([128, J2], F32, tag="ta")
        nc.vector.tensor_tensor(out=t2[:], in0=d2_ps[it][:], in1=z2b[:], op=ADD)
        r2 = sb2.tile([128, J2], F32, tag="tb")
        nc.scalar.activation(r2[:], t2[:], RELU)
        df = sb2.tile([128, J2], F32, tag="tc")
        nc.vector.tensor_tensor(out=df[:], in0=rz2b[:], in1=r2[:], op=SUB)
        prod = sb2.tile([128, J2], F32, tag="td")
        nc.vector.tensor_tensor(out=prod[:], in0=df[:], in1=w3cb[:], op=MULT)
        nc.vector.tensor_reduce(out=numt[:, it:it + 1], in_=prod[:],
                                axis=mybir.AxisListType.X, op=ADD)
        if debug and it == 0:
            d2dbg = sb2.tile([128, J2], F32, tag="d2dbg")
            nc.vector.tensor_copy(d2dbg[:], d2_ps[it][:])
            nc.sync.dma_start(out=t["d2_dbg"][:, :], in_=d2dbg[:])

    num_ps = ps.tile([1, C], F32, tag="scratch")
    for it in range(4):
        nc.tensor.transpose(num_ps[:, 128 * it:128 * (it + 1)],
                            numt[:, it:it + 1], ident[:])
    numr = sb.tile([1, C], F32, tag="numr")
    nc.scalar.copy(numr[:], num_ps[:])
    nc.sync.dma_start(out=t["nm_in"][:, :], in_=numr[:])
    nc.gpsimd.collective_compute(
        "AllReduce", ADD,
        replica_groups=[list(range(NCORES))],
        ins=[t["nm_in"][:, :].opt()], outs=[t["nm_out"][:, :].opt()])
    nc.sync.dma_start(out=t["numer_out"][:, :], in_=t["nm_out"][:, :])
    es.close()


def _build(debug=False, stage=99):
    key = ("nc", debug, stage)
    if key in _CACHE:
        return _CACHE[key]
    nc = bacc.Bacc("TRN2", target_bir_lowering=False, debug=False)
    t = {}
    t["w1t"] = nc.dram_tensor("w1t", [K, J1], F32, kind="ExternalInput")
    t["fch"] = nc.dram_tensor("fch", [128, K // 128], F32, kind="ExternalInput")
    t["b1s"] = nc.dram_tensor("b1s", [1, J1], F32, kind="ExternalInput")
    t["w2t"] = nc.dram_tensor("w2t", [FC, J2], F32, kind="ExternalInput")
    t["b2s"] = nc.dram_tensor("b2s", [1, J2], F32, kind="ExternalInput")
    t["w3n"] = nc.dram_tensor("w3n", [NCLS, J2], F32, kind="ExternalInput")
    t["b3f"] = nc.dram_tensor("b3f", [1, NCLS], F32, kind="ExternalInput")
    t["numer_out"] = nc.dram_tensor("numer_out", [1, C], F32,
                                    kind="ExternalOutput")
    t["logits_out"] = nc.dram_tensor("logits_out", [1, NCLS], F32,
                                     kind="ExternalOutput")
    if debug:
        for name, shape in [("z1_dbg", [1, J1]), ("delta_dbg", [128, J1]),
                            ("h2_dbg", [1, J2]), ("d2_dbg", [128, J2])]:
            t[name] = nc.dram_tensor(name, shape, F32, kind="ExternalOutput")
    t["d1nt_dram"] = nc.dram_tensor("d1nt_dram", [J1, C], F32)
    t["d1full_dram"] = nc.dram_tensor("d1full_dram", [FC, C], F32,
                                      addr_space="Shared")
    t["h1_dram"] = nc.dram_tensor("h1_dram", [1, J1], F32)
    t["h1full_dram"] = nc.dram_tensor("h1full_dram", [NCORES, J1], F32,
                                      addr_space="Shared")
    t["lg_in"] = nc.dram_tensor("lg_in", [1, NCLS], F32)
    t["lg_out"] = nc.dram_tensor("lg_out", [1, NCLS], F32, addr_space="Shared")
    t["nm_in"] = nc.dram_tensor("nm_in", [1, C], F32)
    t["nm_out"] = nc.dram_tensor("nm_out", [1, C], F32, addr_space="Shared")

    with tile.TileContext(nc) as tc:
        _emit(nc, tc, t, debug=debug, stage=stage)
    nc.finalize()
    _CACHE[key] = nc
    return nc


def _maxpool_flat(act):
    a = act[0]
    mp = np.maximum(np.maximum(a[:, 0::2, 0::2], a[:, 0::2, 1::2]),
                    np.maximum(a[:, 1::2, 0::2], a[:, 1::2, 1::2]))
    return mp.reshape(-1).astype(np.float32)  # [25088] channel-major


def _resize_mat(n_in, n_out):
    """Row-stochastic triangle-kernel resize matrix matching
    jax.image.resize(method='bilinear') for upsampling."""
    scale = n_out / n_in
    sample_f = (np.arange(n_out, dtype=np.float64) + 0.5) / scale - 0.5
    x = np.abs(sample_f[None, :] - np.arange(n_in, dtype=np.float64)[:, None])
    w = np.maximum(0.0, 1.0 - x)
    tot = w.sum(axis=0, keepdims=True)
    w = np.where(np.abs(tot) > 1e-9, w / tot, 0.0)
    valid = (sample_f >= -0.5) & (sample_f <= n_in - 0.5)
    w = np.where(valid[None, :], w, 0.0)
    return w.astype(np.float32)  # [n_in, n_out]


def _run_device(inputs, debug=False, stage=99):
    nc = _build(debug=debug, stage=stage)
    W1 = np.asarray(inputs["W1"], dtype=np.float32)
    W2 = np.asarray(inputs["W2"], dtype=np.float32)
    W3 = np.asarray(inputs["W3"], dtype=np.float32)
    b1 = np.asarray(inputs["b1"], dtype=np.float32)
    b2 = np.asarray(inputs["b2"], dtype=np.float32)
    b3 = np.asarray(inputs["b3"], dtype=np.float32).reshape(1, NCLS)
    f = _maxpool_flat(np.asarray(inputs["activations"], dtype=np.float32))
    fch = np.ascontiguousarray(f.reshape(K // 128, 128).T)  # [128, 196]

    in_maps = []
    for m in range(NCORES):
        j1 = slice(J1 * m, J1 * (m + 1))
        j2 = slice(J2 * m, J2 * (m + 1))
        in_maps.append({
            "w1t": np.ascontiguousarray(W1[j1, :].T),
            "fch": fch,
            "b1s": b1[j1].reshape(1, J1),
            "w2t": np.ascontiguousarray(W2[j2, :].T),
            "b2s": b2[j2].reshape(1, J2),
            "w3n": np.ascontiguousarray(W3[:, j2]),
            "b3f": b3,
        })
    return run_bass_kernel_spmd(nc, in_maps, list(range(NCORES)))


def kernel(activations, W1, b1, W2, b2, W3, b3, ht, wt, _debug=False):
    res = _run_device(
        {"activations": activations, "W1": W1, "b1": b1, "W2": W2,
         "b2": b2, "W3": W3, "b3": b3}, debug=_debug)
    out0 = res.results[0]
    logits = out0["logits_out"].reshape(NCLS)
    numer = out0["numer_out"].reshape(C)

    img_class = int(np.argmax(logits))
    base = logits[img_class]
    alpha = (numer / base).astype(np.float32)

    act = np.asarray(activations, dtype=np.float32)[0]      # [512, 14, 14]
    weighted = np.einsum("c,chw->hw", alpha, act).astype(np.float32)

    ht_i, wt_i = int(ht), int(wt)
    rv = _resize_mat(weighted.shape[0], ht_i)                # [14, ht]
    rh = _resize_mat(weighted.shape[1], wt_i)                # [14, wt]
    sal = (rv.T @ weighted @ rh).astype(np.float32)
    fused = np.maximum(sal, 0.0)
    mn, mx = fused.min(), fused.max()
    if mx != mn:
        fused = ((fused - mn) / (mx - mn)).astype(np.float32)
    if _debug:
        kernel._last_debug = dict(out0)
    return fused, np.int32(img_class)


# revision 16
# speedup vs baseline: 18768.8349x; 18768.8349x over previous
"""AblationCAM Trainium2 kernel (8 NeuronCores, SPMD via bass/Tile).

Math restructure (identical function to the reference):
  - maxpool commutes with channel ablation (inputs are non-negative), so
    ablated flat vectors are f with segment i (49 values) zeroed.
  - h1_abl[i] = relu(z1 - delta_i), delta[i, j1] = sum_{k in seg_i} W1T[k, j1] f[k]
  - only column `img_class` of the ablated logits is needed:
      numer[i] = w3c . (relu(z2) - relu(z2 - D2[i])),  D2 = D1 @ W2^T
      alpha[i] = numer[i] / base
  - the saliency fuse/resize/normalize is a tiny (14x14 -> 224x224) linear map
    done on the host, exactly replicating jax.image.resize bilinear weights.

Sharding: W1^T row-sharded over j1 (each core reads its 51.4MB W1 slice once);
W2^T/W3 column-sharded over j2; D1 (8.4MB) AllGathered; logits/numer
AllReduced (tiny).
"""

from contextlib import ExitStack

import numpy as np

import concourse.bacc as bacc
import concourse.bass as bass
import concourse.mybir as mybir
import concourse.tile as tile
from concourse.bass_utils import run_bass_kernel_spmd

F32 = mybir.dt.float32
RELU = mybir.ActivationFunctionType.Relu
ADD = mybir.AluOpType.add
SUB = mybir.AluOpType.subtract
MULT = mybir.AluOpType.mult

NCORES = 8
C = 512            # channels == number of ablations
S = 49             # 7*7 flattened spatial per channel
K = C * S          # 25088 contraction dim of layer 1
FC = 4096
NCLS = 1000
J1 = FC // NCORES  # 512: per-core layer-1 rows
J2 = FC // NCORES  # 512: per-core layer-2 outputs
IB = 128           # channels per i-block group
NG = C // IB       # 4 groups
GT = S             # 49 k-chunks (of 128 rows) per group
SUP = 7            # k-chunks per w1t super-tile DMA
NU = FC // 128     # 32 j1-chunks

_CACHE = {}


def _emit(nc, tc, t, debug=False, stage=99, iters=1):
    fake_comm = (stage == 90)
    if fake_comm:
        stage = 99
    es = ExitStack()
    if iters > 1:
        es.enter_context(tc.For_i(0, iters, 1))
    sb = es.enter_context(tc.tile_pool(name="sb", bufs=1))
    sb2 = es.enter_context(tc.tile_pool(name="sb2", bufs=2))
    sb3 = es.enter_context(tc.tile_pool(name="sb3", bufs=3))
    # PSUM budget is 8 banks of [128, 512]f32:
    #   scratch x2 + acc x2 + d2 x4
    ps = es.enter_context(tc.tile_pool(name="ps", bufs=2, space="PSUM"))
    psacc = es.enter_context(tc.tile_pool(name="psacc", bufs=2, space="PSUM"))
    psd2 = es.enter_context(tc.tile_pool(name="psd2", bufs=1, space="PSUM"))

    # ---------------- constants ----------------
    ident = sb.tile([128, 128], F32, tag="ident")
    nc.sync.dma_start(out=ident[:], in_=t["ident"][:, :])
    mask = sb.tile([128, GT * IB], F32, tag="mask")
    nc.sync.dma_start(out=mask[:], in_=t["mask"][:, :])
    ones_col = sb.tile([128, 1], F32, tag="ones_col")
    nc.vector.memset(ones_col[:], 1.0)
    ones_row = sb.tile([1, 128], F32, tag="ones_row")
    nc.vector.memset(ones_row[:], 1.0)
    one_one = sb.tile([1, 1], F32, tag="one_one")
    nc.vector.memset(one_one[:], 1.0)

    # ---------------- small inputs ----------------
    fch = sb.tile([128, K // 128], F32, tag="fch")       # f[128u + p] at [p, u]
    nc.sync.dma_start(out=fch[:], in_=t["fch"][:, :])
    b1s = sb.tile([1, J1], F32, tag="b1s")
    nc.sync.dma_start(out=b1s[:], in_=t["b1s"][:, :])
    b2s = sb.tile([1, J2], F32, tag="b2s")
    nc.sync.dma_start(out=b2s[:], in_=t["b2s"][:, :])
    b3f = sb.tile([1, NCLS], F32, tag="b3f")
    nc.sync.dma_start(out=b3f[:], in_=t["b3f"][:, :])
    # W3 natural [1000, 512]: 8 row-tiles (c on partitions)
    w3rows = [(128 * q, min(128, NCLS - 128 * q)) for q in range(8)]
    w3n = []
    for q, (r0, pr) in enumerate(w3rows):
        w3n_q = sb.tile([128, J2], F32, tag=f"w3n{q}")
        nc.sync.dma_start(out=w3n_q[:pr, :], in_=t["w3n"][r0:r0 + pr, :])
        w3n.append(w3n_q)

    # ---------------- W1 stage: delta + z1 ----------------
    # sf[p, tt, c] = f[128(49 g + tt) + p] if c == floor((128 tt + p)/49) else 0
    delta_sb = []
    z1_ps = psacc.tile([1, J1], F32, tag="acc")
    for g in range(NG):
        sf = sb.tile([128, GT, IB], F32, tag="sf", name=f"sf{g}")
        fg = fch[:, GT * g:GT * (g + 1)].unsqueeze(2).broadcast_to((128, GT, IB))
        nc.vector.tensor_tensor(
            out=sf[:], in0=mask[:].rearrange("p (t c) -> p t c", c=IB),
            in1=fg, op=MULT)
        d_ps = ps.tile([128, J1], F32, tag="scratch")
        for sup in range(GT // SUP):
            w1sup = sb2.tile([128, SUP, J1], F32, tag="w1sup")
            r0 = 128 * (GT * g + SUP * sup)
            nc.sync.dma_start(
                out=w1sup[:],
                in_=t["w1t"][r0:r0 + 128 * SUP, :].rearrange(
                    "(q p) j -> p q j", p=128, q=SUP))
            for q in range(SUP):
                tt = SUP * sup + q
                nc.tensor.matmul(
                    d_ps[:], sf[:, tt, :], w1sup[:, q, :],
                    start=(tt == 0), stop=(tt == GT - 1))
        d_g = sb.tile([128, J1], F32, tag=f"delta{g}")
        nc.vector.tensor_copy(d_g[:], d_ps[:])
        delta_sb.append(d_g)
        # z1 partial: sum over the 128 channels of this group
        nc.tensor.matmul(z1_ps[:], ones_col[:], d_g[:],
                         start=(g == 0), stop=(g == NG - 1))

    # z1 = sum_i delta + b1 ; h1 = relu(z1)
    z1 = sb.tile([1, J1], F32, tag="z1")
    nc.vector.tensor_tensor(out=z1[:], in0=z1_ps[:], in1=b1s[:], op=ADD)
    h1 = sb.tile([1, J1], F32, tag="h1")
    nc.scalar.activation(h1[:], z1[:], RELU)
    if debug:
        nc.sync.dma_start(out=t["z1_dbg"][:, :], in_=z1[:])
        nc.sync.dma_start(out=t["delta_dbg"][:, :], in_=delta_sb[0][:])

    if stage <= 1:
        es.close()
        return

    # broadcast z1 across partitions: [128, J1]
    z1b_ps = ps.tile([128, J1], F32, tag="scratch")
    nc.tensor.matmul(z1b_ps[:], ones_row[:], z1[:], start=True, stop=True)
    z1b = sb.tile([128, J1], F32, tag="z1b")
    nc.scalar.copy(z1b[:], z1b_ps[:])
    rz1b = sb.tile([128, J1], F32, tag="rz1b")
    nc.scalar.activation(rz1b[:], z1b[:], RELU)

    # d1n[i, j1] = relu(z1 - delta_i) - relu(z1)   (= h1_abl - h1_base)
    # transpose to [j1, i] and write to dram for the allgather
    for g in range(NG):
        t1 = sb2.tile([128, J1], F32, tag="ta")
        nc.vector.tensor_tensor(out=t1[:], in0=z1b[:], in1=delta_sb[g][:], op=SUB)
        r1 = sb2.tile([128, J1], F32, tag="tb")
        nc.scalar.activation(r1[:], t1[:], RELU)
        d1n_g = sb2.tile([128, J1], F32, tag="tc")
        nc.vector.tensor_tensor(out=d1n_g[:], in0=r1[:], in1=rz1b[:], op=SUB)
        for u in range(J1 // 128):
            tp = ps.tile([128, 128], F32, tag="scratch")
            nc.tensor.transpose(tp[:], d1n_g[:, 128 * u:128 * (u + 1)], ident[:])
            ev = sb2.tile([128, 128], F32, tag="tpe")
            nc.scalar.copy(ev[:], tp[:])
            nc.sync.dma_start(
                out=t["d1nt_dram"][128 * u:128 * (u + 1),
                                   128 * g:128 * (g + 1)],
                in_=ev[:])

    if not fake_comm:
        nc.gpsimd.collective_compute(
            "AllGather", mybir.AluOpType.bypass,
            replica_groups=[list(range(NCORES))],
            ins=[t["d1nt_dram"][:, :].opt()],
            outs=[t["d1full_dram"][:, :].opt()])

    # ---------------- h1 gather, z2 pass ----------------
    nc.sync.dma_start(out=t["h1_dram"][:, :], in_=h1[:])
    h1nat = sb.tile([32, 128], F32, tag="h1nat")
    if not fake_comm:
        nc.gpsimd.collective_compute(
            "AllGather", mybir.AluOpType.bypass,
            replica_groups=[list(range(NCORES))],
            ins=[t["h1_dram"][:, :].opt()], outs=[t["h1full_dram"][:, :].opt()])
        nc.sync.dma_start(
            out=h1nat[:],
            in_=t["h1full_dram"][:, :].rearrange("a (b c) -> (a b) c", c=128))
    else:
        nc.vector.memset(h1nat[:], 0.0)
        nc.sync.dma_start(
            out=h1nat[:4, :],
            in_=t["h1_dram"][:, :].rearrange("a (b c) -> (a b) c", c=128))
    h1t_ps = ps.tile([128, 32], F32, tag="scratch")
    nc.tensor.transpose(h1t_ps[:], h1nat[:], ident[:32, :32])
    h1t = sb.tile([128, 32], F32, tag="h1t")
    nc.scalar.copy(h1t[:], h1t_ps[:])

    z2_ps = psacc.tile([1, J2], F32, tag="acc")
    for u in range(NU):
        w2t_u = sb3.tile([128, J2], F32, tag="w2ta")
        nc.sync.dma_start(out=w2t_u[:], in_=t["w2t"][128 * u:128 * (u + 1), :])
        nc.tensor.matmul(z2_ps[:], h1t[:, u:u + 1], w2t_u[:],
                         start=(u == 0), stop=(u == NU - 1))
    z2 = sb.tile([1, J2], F32, tag="z2")
    nc.vector.tensor_tensor(out=z2[:], in0=z2_ps[:], in1=b2s[:], op=ADD)
    z2b_ps = ps.tile([128, J2], F32, tag="scratch")
    nc.tensor.matmul(z2b_ps[:], ones_row[:], z2[:], start=True, stop=True)
    z2b = sb.tile([128, J2], F32, tag="z2b")
    nc.scalar.copy(z2b[:], z2b_ps[:])
    rz2b = sb.tile([128, J2], F32, tag="rz2b")
    nc.scalar.activation(rz2b[:], z2b[:], RELU)
    if debug:
        h2dbg = sb.tile([1, J2], F32, tag="h2dbg")
        nc.scalar.activation(h2dbg[:], z2[:], RELU)
        nc.sync.dma_start(out=t["h2_dbg"][:, :], in_=h2dbg[:])

    if stage <= 2:
        es.close()
        return

    # ---------------- logits via row-dot TTR ----------------
    lgT = sb.tile([128, 8], F32, tag="lgT")
    for q, (r0, pr) in enumerate(w3rows):
        prod = sb2.tile([128, J2], F32, tag="td")
        nc.vector.tensor_tensor(out=prod[:pr, :], in0=w3n[q][:pr, :],
                                in1=rz2b[:pr, :], op=MULT)
        nc.vector.tensor_reduce(out=lgT[:pr, q:q + 1], in_=prod[:pr, :],
                                axis=mybir.AxisListType.X, op=ADD)
    lga = ps.tile([1, 512], F32, tag="scratch")
    lgb_ps = ps.tile([1, 512], F32, tag="scratch")
    for q, (r0, pr) in enumerate(w3rows):
        dst = lga if q < 4 else lgb_ps
        col = 128 * q if q < 4 else 128 * (q - 4)
        nc.tensor.transpose(dst[:, col:col + pr], lgT[:pr, q:q + 1],
                            ident[:pr, :pr])
    lg = sb.tile([1, NCLS], F32, tag="lg")
    nc.scalar.copy(lg[:, 0:512], lga[:])
    nc.scalar.copy(lg[:, 512:NCLS], lgb_ps[:, 0:NCLS - 512])
    nc.sync.dma_start(out=t["lg_in"][:, :], in_=lg[:])
    lgf = sb.tile([1, NCLS], F32, tag="lgf")
    if not fake_comm:
        nc.gpsimd.collective_compute(
            "AllReduce", ADD,
            replica_groups=[list(range(NCORES))],
            ins=[t["lg_in"][:, :].opt()], outs=[t["lg_out"][:, :].opt()])
        nc.sync.dma_start(out=lgf[:], in_=t["lg_out"][:, :])
    else:
        nc.sync.dma_start(out=lgf[:], in_=t["lg_in"][:, :])
    lgb = sb.tile([1, NCLS], F32, tag="lgb")
    nc.vector.tensor_tensor(out=lgb[:], in0=lgf[:], in1=b3f[:], op=ADD)
    nc.sync.dma_start(out=t["logits_out"][:, :], in_=lgb[:])

    # ---------------- one-hot argmax -> w3c ----------------
    mx = sb.tile([1, 1], F32, tag="mx")
    nc.vector.tensor_reduce(out=mx[:], in_=lgb[:], axis=mybir.AxisListType.X,
                            op=mybir.AluOpType.max)
    oh = sb.tile([1, NCLS], F32, tag="oh")
    nc.vector.tensor_scalar(out=oh[:], in0=lgb[:], scalar1=mx[:], scalar2=None,
                            op0=mybir.AluOpType.is_equal)
    ohT_ps = ps.tile([128, 8], F32, tag="scratch")
    for q, (r0, pr) in enumerate(w3rows):
        nc.tensor.transpose(ohT_ps[:pr, q:q + 1], oh[:, r0:r0 + pr], one_one[:])
    ohT = sb.tile([128, 8], F32, tag="ohT")
    for q, (r0, pr) in enumerate(w3rows):
        nc.scalar.copy(ohT[:pr, q:q + 1], ohT_ps[:pr, q:q + 1])
    w3c_ps = psacc.tile([1, J2], F32, tag="acc")
    for q, (r0, pr) in enumerate(w3rows):
        nc.tensor.matmul(w3c_ps[:], ohT[:pr, q:q + 1], w3n[q][:pr, :],
                         start=(q == 0), stop=(q == 7))
    w3c = sb.tile([1, J2], F32, tag="w3c")
    nc.scalar.copy(w3c[:], w3c_ps[:])
    w3cb_ps = ps.tile([128, J2], F32, tag="scratch")
    nc.tensor.matmul(w3cb_ps[:], ones_row[:], w3c[:], start=True, stop=True)
    w3cb = sb.tile([128, J2], F32, tag="w3cb")
    nc.scalar.copy(w3cb[:], w3cb_ps[:])

    if stage <= 3:
        es.close()
        return

    # ---------------- D2 pass ----------------
    d2_ps = [psd2.tile([128, J2], F32, tag=f"d2ps{it}", name=f"d2ps{it}")
             for it in range(4)]
    for u in range(NU):
        d1f_u = sb3.tile([128, C], F32, tag="d1f")
        d1src = (t["d1full_dram"][128 * u:128 * (u + 1), :] if not fake_comm
                 else t["d1nt_dram"][128 * (u % 4):128 * (u % 4 + 1), :])
        nc.sync.dma_start(out=d1f_u[:], in_=d1src)
        w2t_u = sb3.tile([128, J2], F32, tag="w2tb")
        nc.sync.dma_start(out=w2t_u[:], in_=t["w2t"][128 * u:128 * (u + 1), :])
        for it in range(4):
            nc.tensor.matmul(d2_ps[it][:], d1f_u[:, 128 * it:128 * (it + 1)],
                             w2t_u[:], start=(u == 0), stop=(u == NU - 1))

    # ---------------- R + numer ----------------
    numt = sb.tile([128, 4], F32, tag="numt")
    for it in range(4):
        t2 = sb2.tile([128, J2], F32, tag="ta")
        nc.vector.tensor_tensor(out=t2[:], in0=d2_ps[it][:], in1=z2b[:], op=ADD)
        r2 = sb2.tile([128, J2], F32, tag="tb")
        nc.scalar.activation(r2[:], t2[:], RELU)
        df = sb2.tile([128, J2], F32, tag="tc")
        nc.vector.tensor_tensor(out=df[:], in0=rz2b[:], in1=r2[:], op=SUB)
        prod = sb2.tile([128, J2], F32, tag="td")
        nc.vector.tensor_tensor(out=prod[:], in0=df[:], in1=w3cb[:], op=MULT)
        nc.vector.tensor_reduce(out=numt[:, it:it + 1], in_=prod[:],
                                axis=mybir.AxisListType.X, op=ADD)
        if debug and it == 0:
            d2dbg = sb2.tile([128, J2], F32, tag="d2dbg")
            nc.vector.tensor_copy(d2dbg[:], d2_ps[it][:])
            nc.sync.dma_start(out=t["d2_dbg"][:, :], in_=d2dbg[:])

    num_ps = ps.tile([1, C], F32, tag="scratch")
    for it in range(4):
        nc.tensor.transpose(num_ps[:, 128 * it:128 * (it + 1)],
                            numt[:, it:it + 1], ident[:])
    numr = sb.tile([1, C], F32, tag="numr")
    nc.scalar.copy(numr[:], num_ps[:])
    nc.sync.dma_start(out=t["nm_in"][:, :], in_=numr[:])
    if not fake_comm:
        nc.gpsimd.collective_compute(
            "AllReduce", ADD,
            replica_groups=[list(range(NCORES))],
            ins=[t["nm_in"][:, :].opt()], outs=[t["nm_out"][:, :].opt()])
        nc.sync.dma_start(out=t["numer_out"][:, :], in_=t["nm_out"][:, :])
    else:
        nc.sync.dma_start(out=t["numer_out"][:, :], in_=t["nm_in"][:, :])
    es.close()


def _build(debug=False, stage=99, iters=1):
    key = ("nc", debug, stage, iters)
    if key in _CACHE:
        return _CACHE[key]
    nc = bacc.Bacc("TRN2", target_bir_lowering=False, debug=False)
    t = {}
    t["w1t"] = nc.dram_tensor("w1t", [K, J1], F32, kind="ExternalInput")
    t["ident"] = nc.dram_tensor("ident", [128, 128], F32, kind="ExternalInput")
    t["mask"] = nc.dram_tensor("mask", [128, GT * IB], F32,
                               kind="ExternalInput")
    t["fch"] = nc.dram_tensor("fch", [128, K // 128], F32, kind="ExternalInput")
    t["b1s"] = nc.dram_tensor("b1s", [1, J1], F32, kind="ExternalInput")
    t["w2t"] = nc.dram_tensor("w2t", [FC, J2], F32, kind="ExternalInput")
    t["b2s"] = nc.dram_tensor("b2s", [1, J2], F32, kind="ExternalInput")
    t["w3n"] = nc.dram_tensor("w3n", [NCLS, J2], F32, kind="ExternalInput")
    t["b3f"] = nc.dram_tensor("b3f", [1, NCLS], F32, kind="ExternalInput")
    t["numer_out"] = nc.dram_tensor("numer_out", [1, C], F32,
                                    kind="ExternalOutput")
    t["logits_out"] = nc.dram_tensor("logits_out", [1, NCLS], F32,
                                     kind="ExternalOutput")
    if debug:
        for name, shape in [("z1_dbg", [1, J1]), ("delta_dbg", [128, J1]),
                            ("h2_dbg", [1, J2]), ("d2_dbg", [128, J2])]:
            t[name] = nc.dram_tensor(name, shape, F32, kind="ExternalOutput")
    t["d1nt_dram"] = nc.dram_tensor("d1nt_dram", [J1, C], F32)
    t["d1full_dram"] = nc.dram_tensor("d1full_dram", [FC, C], F32,
                                      addr_space="Shared")
    t["h1_dram"] = nc.dram_tensor("h1_dram", [1, J1], F32)
    t["h1full_dram"] = nc.dram_tensor("h1full_dram", [NCORES, J1], F32,
                                      addr_space="Shared")
    t["lg_in"] = nc.dram_tensor("lg_in", [1, NCLS], F32)
    t["lg_out"] = nc.dram_tensor("lg_out", [1, NCLS], F32, addr_space="Shared")
    t["nm_in"] = nc.dram_tensor("nm_in", [1, C], F32)
    t["nm_out"] = nc.dram_tensor("nm_out", [1, C], F32, addr_space="Shared")

    with tile.TileContext(nc) as tc:
        _emit(nc, tc, t, debug=debug, stage=stage, iters=iters)
    nc.finalize()
    _CACHE[key] = nc
    return nc


def _maxpool_flat(act):
    a = act[0]
    mp = np.maximum(np.maximum(a[:, 0::2, 0::2], a[:, 0::2, 1::2]),
                    np.maximum(a[:, 1::2, 0::2], a[:, 1::2, 1::2]))
    return mp.reshape(-1).astype(np.float32)  # [25088] channel-major


def _resize_mat(n_in, n_out):
    """Row-stochastic triangle-kernel resize matrix matching
    jax.image.resize(method='bilinear') for upsampling."""
    scale = n_out / n_in
    sample_f = (np.arange(n_out, dtype=np.float64) + 0.5) / scale - 0.5
    x = np.abs(sample_f[None, :] - np.arange(n_in, dtype=np.float64)[:, None])
    w = np.maximum(0.0, 1.0 - x)
    tot = w.sum(axis=0, keepdims=True)
    w = np.where(np.abs(tot) > 1e-9, w / tot, 0.0)
    valid = (sample_f >= -0.5) & (sample_f <= n_in - 0.5)
    w = np.where(valid[None, :], w, 0.0)
    return w.astype(np.float32)  # [n_in, n_out]


def _make_in_maps(inputs):
    W1 = np.asarray(inputs["W1"], dtype=np.float32)
    W2 = np.asarray(inputs["W2"], dtype=np.float32)
    W3 = np.asarray(inputs["W3"], dtype=np.float32)
    b1 = np.asarray(inputs["b1"], dtype=np.float32)
    b2 = np.asarray(inputs["b2"], dtype=np.float32)
    b3 = np.asarray(inputs["b3"], dtype=np.float32).reshape(1, NCLS)
    f = _maxpool_flat(np.asarray(inputs["activations"], dtype=np.float32))
    fch = np.ascontiguousarray(f.reshape(K // 128, 128).T)  # [128, 196]
    ident_np = np.eye(128, dtype=np.float32)
    pp = np.arange(128)[:, None, None]
    ttt = np.arange(GT)[None, :, None]
    cc = np.arange(IB)[None, None, :]
    mask_np = np.ascontiguousarray(
        (((128 * ttt + pp) // 49) == cc).astype(np.float32).reshape(128, GT * IB))

    in_maps = []
    for m in range(NCORES):
        j1 = slice(J1 * m, J1 * (m + 1))
        j2 = slice(J2 * m, J2 * (m + 1))
        in_maps.append({
            "w1t": np.ascontiguousarray(W1[j1, :].T),
            "fch": fch,
            "ident": ident_np,
            "mask": mask_np,
            "b1s": b1[j1].reshape(1, J1),
            "w2t": np.ascontiguousarray(W2[j2, :].T),
            "b2s": b2[j2].reshape(1, J2),
            "w3n": np.ascontiguousarray(W3[:, j2]),
            "b3f": b3,
        })
    return in_maps


def _run_device(inputs, debug=False, stage=99):
    nc = _build(debug=debug, stage=stage)
    in_maps = _make_in_maps(inputs)
    return run_bass_kernel_spmd(nc, in_maps, list(range(NCORES)))


def kernel(activations, W1, b1, W2, b2, W3, b3, ht, wt, _debug=False):
    res = _run_device(
        {"activations": activations, "W1": W1, "b1": b1, "W2": W2,
         "b2": b2, "W3": W3, "b3": b3}, debug=_debug)
    out0 = res.results[0]
    logits = out0["logits_out"].reshape(NCLS)
    numer = out0["numer_out"].reshape(C)

    img_class = int(np.argmax(logits))
    base = logits[img_class]
    alpha = (numer / base).astype(np.float32)

    act = np.asarray(activations, dtype=np.float32)[0]      # [512, 14, 14]
    weighted = np.einsum("c,chw->hw", alpha, act).astype(np.float32)

    ht_i, wt_i = int(ht), int(wt)
    rv = _resize_mat(weighted.shape[0], ht_i)                # [14, ht]
    rh = _resize_mat(weighted.shape[1], wt_i)                # [14, wt]
    sal = (rv.T @ weighted @ rh).astype(np.float32)
    fused = np.maximum(sal, 0.0)
    mn, mx = fused.min(), fused.max()
    if mx != mn:
        fused = ((fused - mn) / (mx - mn)).astype(np.float32)
    if _debug:
        kernel._last_debug = dict(out0)
    return fused, np.int32(img_class)
